# revision 86
# baseline (speedup 1.0000x reference)
# Trainium2 Bass kernel for nn_MultiHeadAttention_24902220382931.
#
# Strategy: data-parallel over sentences. The 32 variable-length sentences are
# sorted by length; core c processes ranks {c, 15-c, 16+c, 31-c} (exactly equal
# token counts, near-equal attention work). Each core packs its 4 sentences
# into 4 fixed-size slots (max length per slot across cores, regions rounded to
# 128) so that all 8 cores execute one identical SPMD program.
#
# v2: fp8(e4m3) DoubleRow matmuls everywhere on PE (2 contraction planes per
# instruction), transposed-attention dataflow:
#   scoresT[k,q] = K^T-chunk @ Q  (dk=128 contracted as 2x64 DoubleRow planes)
#   attn = exp(scoresT*scale + bias_row)    (pad-key rows get bias -30 -> 0)
#   bc[p,q] = ones^T @ attn = sum_k attn    (denominator, broadcast to 128
#                                            partitions for free by the ones
#                                            lhsT having 128 columns)
#   ot[d,q] = V^T-chunks @ attn             (unnormalized output)
#   oo = ot / bc                            (DVE/Pool divide, writes the
#                                            proj-ready layout directly)
# This removes the attn^T transpose matmuls, the per-q-chunk diag build, the
# PSUM->SBUF attn copies and the softmax-sum fixup chain of v1.
#
# Scaling for fp8 dynamic range: all weights are scaled by 32 on the host
# (xavier std ~0.008 would be subnormal in e4m3). logits come out x1024 ->
# exp scale 1/(32*1024). v = 32*v_true -> oo = ot/bc = 32*o_true (good fp8
# range ~0.3). proj weights x32 -> z_psum = 1024*z_true; the residual x is
# pre-scaled x1024 on the host and the z-add applies a 1/1024 output scale.
import sys

for _p in ("/opt/trn_rl_repo", "/root/.axon_site/_ro/trn_rl_repo"):
    if _p not in sys.path:
        sys.path.insert(0, _p)

import numpy as np
import ml_dtypes

import concourse.bass as bass  # noqa: F401
import concourse.mybir as mybir
import concourse.tile as tile
from concourse import bacc

BF16 = ml_dtypes.bfloat16
FP8 = ml_dtypes.float8_e4m3
F32 = np.float32

N_CORES = 8
MB = 32
D_MODEL = 1024
D_HALF = 512
N_HEAD = 8
D_K = 128
SCALE = float(D_MODEL) ** 0.5  # 32.0
W_SCALE = 32.0                 # host-side fp8 weight scale
X_SCALE = 1024.0               # host-side residual scale (W_SCALE**2)
EXP_SCALE = 1.0 / (SCALE * W_SCALE * W_SCALE)
EPS = 1e-3
P = 128


def _ceil_to(x, m):
    return (x + m - 1) // m * m


class Plan:
    def __init__(self, lengths):
        lengths = np.asarray(lengths, np.int64)
        assert lengths.shape == (MB,)
        order = np.argsort(-lengths, kind="stable")
        self.core_sents = [
            [int(order[c]), int(order[15 - c]), int(order[16 + c]), int(order[31 - c])]
            for c in range(N_CORES)
        ]
        self.lengths = lengths
        self.slot_pad = [
            max(int(lengths[self.core_sents[c][j]]) for c in range(N_CORES))
            for j in range(4)
        ]
        self.regions = [_ceil_to(sp, P) for sp in self.slot_pad]
        self.offs = [0]
        for r in self.regions[:-1]:
            self.offs.append(self.offs[-1] + r)
        self.t_pad = sum(self.regions)
        assert self.t_pad % P == 0
        self.nt = self.t_pad // P
        self.glob_off = np.concatenate([[0], np.cumsum(lengths)[:-1]]).astype(np.int64)
        # all chunks below the last must be fully valid on every core
        for s in range(4):
            nk = self.regions[s] // P
            min_l = min(int(lengths[self.core_sents[c][s]]) for c in range(N_CORES))
            assert min_l >= P * (nk - 1), (s, min_l, nk)

    @property
    def key(self):
        return (tuple(self.slot_pad), self.t_pad)


DR = mybir.MatmulPerfMode.DoubleRow


def _build_program(plan: Plan, loop_n: int = 1):
    """Build and compile the single-core Bass program (same for all cores)."""
    import contextlib
    T = plan.t_pad
    nc = bacc.Bacc("TRN2", target_bir_lowering=False, debug=False)

    dt = mybir.dt
    # ---- DRAM I/O ----
    xT_d = nc.dram_tensor("xT", [D_MODEL, T], dt.float8e4, kind="ExternalInput").ap()
    x_d = nc.dram_tensor("x", [T, D_MODEL], dt.bfloat16, kind="ExternalInput").ap()
    wq_d = nc.dram_tensor("wq", [P, 4 * 8 * P], dt.float8e4, kind="ExternalInput").ap()
    wk_d = nc.dram_tensor("wk", [P, 4 * 8 * P], dt.float8e4, kind="ExternalInput").ap()
    wv_d = nc.dram_tensor("wv", [P, 8 * D_HALF], dt.float8e4, kind="ExternalInput").ap()
    pw_d = nc.dram_tensor("pw", [P, 8 * D_HALF], dt.float8e4, kind="ExternalInput").ap()
    bias_d = nc.dram_tensor("bias", [P, 4], dt.float32, kind="ExternalInput").ap()
    out_d = nc.dram_tensor("out", [T, D_MODEL], dt.bfloat16,
                           kind="ExternalOutput").ap()

    with tile.TileContext(nc) as tc:
        with (
            tc.tile_pool(name="persist", bufs=1) as pp,
            tc.tile_pool(name="weights", bufs=1) as wp,
        ):
            # qt/kt: [p(2 heads x 64 dk-of-comp), comp, pair, region] fp8
            qt = [pp.tile([P, 2, 4, r], dt.float8e4, name=f"qt{s}", tag=f"qt{s}")
                  for s, r in enumerate(plan.regions)]
            kt = [pp.tile([P, 2, 4, r], dt.float8e4, name=f"kt{s}", tag=f"kt{s}")
                  for s, r in enumerate(plan.regions)]
            # V token-natural: [p, tile, head*128 + half*64 + d]
            vv = [pp.tile([P, r // P, D_MODEL], dt.float8e4, name=f"vv{s}",
                          tag=f"vv{s}")
                  for s, r in enumerate(plan.regions)]
            # attention out, proj-ready: [64*(h%2)+d, i(v1/v2), h//2, token]
            oo = [pp.tile([P, 2, 4, r], dt.float8e4, name=f"oo{s}",
                          tag=f"oo{s}")
                  for s, r in enumerate(plan.regions)]
            bias_sb = pp.tile([P, 4], dt.float32, tag="bias")
            ones_sb = pp.tile([P, 2, P], dt.float8e4, tag="ones")

            wq_sb = wp.tile([P, 4, 8, P], dt.float8e4, tag="wq")
            wk_sb = wp.tile([P, 4, 8, P], dt.float8e4, tag="wk")
            wv_sb = wp.tile([P, 8, D_HALF], dt.float8e4, tag="wv")
            pw_sb = wp.tile([P, 2, 2, 2, D_HALF], dt.float8e4, tag="pw")

            nc.gpsimd.dma_start(bias_sb[:, :], bias_d[:, :])
            nc.scalar.dma_start(
                wq_sb.rearrange("p a b c -> p (a b c)")[:, :], wq_d[:, :])
            nc.scalar.dma_start(
                wk_sb.rearrange("p a b c -> p (a b c)")[:, :], wk_d[:, :])
            nc.gpsimd.dma_start(
                wv_sb.rearrange("p a b -> p (a b)")[:, :], wv_d[:, :])
            nc.scalar.dma_start(
                pw_sb.rearrange("p a b c d -> p (a b c d)")[:, :], pw_d[:, :])
            nc.gpsimd.memset(ones_sb[:, :, :], 1.0)

            loop_cm = (tc.For_i(0, loop_n, 1,
                                hint_engines=(mybir.EngineType.PE,
                                              mybir.EngineType.DVE,
                                              mybir.EngineType.Activation,
                                              mybir.EngineType.SP))
                       if loop_n > 1 else contextlib.nullcontext())
            with loop_cm:
                _kernel_body(nc, tc, plan, locals())

    nc.compile()
    return nc


def _kernel_body(nc, tc, plan, env):
    dt = mybir.dt
    qt, kt, vv, oo = env["qt"], env["kt"], env["vv"], env["oo"]
    bias_sb, ones_sb = env["bias_sb"], env["ones_sb"]
    wq_sb, wk_sb, wv_sb, pw_sb = (env["wq_sb"], env["wk_sb"], env["wv_sb"],
                                  env["pw_sb"])
    xT_d, x_d, out_d = env["xT_d"], env["x_d"], env["out_d"]
    Exp = mybir.ActivationFunctionType.Exp
    Square = mybir.ActivationFunctionType.Square
    Sqrt = mybir.ActivationFunctionType.Sqrt
    add_op = mybir.AluOpType.add
    sub_op = mybir.AluOpType.subtract
    mult_op = mybir.AluOpType.mult
    div_op = mybir.AluOpType.divide

    # zero the pad-query tails of the attention output staging (never written
    # by normalize; proj matmuls read the full region)
    for s in range(4):
        L, r = plan.slot_pad[s], plan.regions[s]
        if r > L:
            nc.gpsimd.memset(oo[s][:, :, :, L:r], 0.0)

    # ============ QKV projections, then per-slot attention + proj/LN =======
    # PSUM budget (8 banks): sc pool = 3 x [128,2,512] tiles (6 banks) through
    # which scores pairs, the ot/bc combo AND the proj z accumulators all
    # rotate; acc pool (QKV) = 2 banks.
    with (
        tc.tile_pool(name="xt_pool", bufs=4) as xtp,
        tc.tile_pool(name="sc_ps", bufs=3, space="PSUM") as scp,
        tc.tile_pool(name="attn_sb", bufs=3) as asb,
        tc.tile_pool(name="rec_sb", bufs=2) as recp,
    ):
        env["recp"] = recp
        # ---- QKV ----
        ev_rot = 0
        ev_engs = (nc.vector, nc.scalar)
        with tc.tile_pool(name="acc_ps", bufs=2, space="PSUM") as accp:
            for s in range(4):
                gw = plan.regions[s]
                g0 = plan.offs[s]
                xt_sb = xtp.tile([P, 8, D_HALF], dt.float8e4, tag="xt")
                xT_r = xT_d.rearrange("(c p) t -> p c t", p=P)
                nc.sync.dma_start(xt_sb[:, :, 0:gw], xT_r[:, :, g0:g0 + gw])
                # pr0 Q/K first (unblocks heads 0-1's scores), then all of V
                # (every head's attn@V needs the whole slot's vv), then the
                # remaining Q/K pairs
                def emit_qk(pr):
                    for w_sb, dst in ((wq_sb, qt), (wk_sb, kt)):
                        # scores read only query cols 0:L of qt; kt needs the
                        # full region (whole key chunks)
                        cw = plan.slot_pad[s] if dst is qt else gw
                        for half in range(2):
                            acc = accp.tile([P, D_HALF], dt.float32,
                                            tag="acc", name="acc")
                            for jp in range(2):
                                j = half * 4 + jp * 2
                                nc.tensor.matmul(
                                    acc[:, 0:cw],
                                    w_sb[:, pr, j:j + 2, :],
                                    xt_sb[:, j:j + 2, 0:cw],
                                    start=(jp == 0),
                                    stop=(jp == 1),
                                    perf_mode=DR,
                                )
                            nonlocal ev_rot
                            eng = ev_engs[ev_rot % len(ev_engs)]
                            ev_rot += 1
                            off = None if eng is nc.vector else -(10 ** 6)
                            with tc.high_priority(offset=off):
                                if eng is nc.scalar:
                                    eng.copy(dst[s][:, half, pr, 0:cw],
                                             acc[:, 0:cw])
                                else:
                                    eng.tensor_copy(dst[s][:, half, pr, 0:cw],
                                                    acc[:, 0:cw])

                emit_qk(0)
                vv_w = vv[s].rearrange("p t (h b d) -> p t h b d",
                                       h=N_HEAD, b=2)
                for tt in range(gw // P):
                    tl = tt * P
                    for half in range(2):
                        vacc = accp.tile([P, D_HALF], dt.float32,
                                         tag="acc", name="vacc")
                        vacc_r = vacc.rearrange("p (h d) -> p h d", h=N_HEAD)
                        for jp in range(2):
                            j = half * 4 + jp * 2
                            nc.tensor.matmul(
                                vacc[:, :],
                                xt_sb[:, j:j + 2, tl:tl + P],
                                wv_sb[:, j:j + 2, :],
                                start=(jp == 0),
                                stop=(jp == 1),
                                perf_mode=DR,
                            )
                        eng = ev_engs[ev_rot % len(ev_engs)]
                        ev_rot += 1
                        off = None if eng is nc.vector else -(10 ** 6)
                        with tc.high_priority(offset=off):
                            if eng is nc.scalar:
                                eng.copy(vv_w[:, tt, :, half, :],
                                         vacc_r[:, :, :])
                            else:
                                eng.tensor_copy(vv_w[:, tt, :, half, :],
                                                vacc_r[:, :, :])
                for pr in range(1, 4):
                    emit_qk(pr)

        # ---- per slot: attention heads with proj tiles of the previous
        # slot interleaved between head pairs (they share the sc rotation) --
        with (
            tc.tile_pool(name="x_sb", bufs=2) as xsb,
            tc.tile_pool(name="z_sb", bufs=16) as zsb,
            tc.tile_pool(name="o_sb", bufs=2) as osb,
            tc.tile_pool(name="scr_sb", bufs=2) as scrp,
            tc.tile_pool(name="st_sb", bufs=2) as stp,
        ):
            pools = (scp, xsb, zsb, osb, scrp, stp)
            env["zs_g"] = stp.tile([P, plan.nt], mybir.dt.float32,
                                   tag="zs", bufs=1, name="zs_g")
            env["sq_g"] = stp.tile([P, plan.nt], mybir.dt.float32,
                                   tag="sq", bufs=1, name="sq_g")
            env["zt_g"] = [None] * plan.nt
            env["x_slot"] = [None] * 4
            prev = []
            for s in range(4):
                it = iter(prev)
                for pr in range(4):
                    _attn_pair(nc, tc, plan, env, s, pr, scp, asb)
                    fn = next(it, None)
                    if fn is not None:
                        fn()
                for fn in it:
                    fn()
                prev = _proj_ln_closures(nc, tc, plan, env, s, pools)
                if s == 2:
                    # slots 0-1 proj tiles have drained; their stats +
                    # applies overlap slot-3 attention
                    _final_stats(nc, tc, plan, env, pools, 0,
                                 plan.offs[2] // P)
            for fn in prev:
                fn()
            _final_stats(nc, tc, plan, env, pools,
                         plan.offs[2] // P, plan.offs[3] // P)
            _final_stats(nc, tc, plan, env, pools,
                         plan.offs[3] // P, plan.nt)


def _attn_pair(nc, tc, plan, env, s, pr, scp, asb):
    """Attention for the head pair (2*pr, 2*pr+1) of slot s.

    scores/exp run per head; the attn@V and denominator matmuls write M=64
    groups into shared pair tiles so ONE [128, 2, L] DVE divide normalizes
    and distributes both heads (GPSIMD may not touch PSUM on real HW)."""
    dt = mybir.dt
    qt, kt, vv, oo = env["qt"], env["kt"], env["vv"], env["oo"]
    bias_sb, ones_sb = env["bias_sb"], env["ones_sb"]
    Exp = mybir.ActivationFunctionType.Exp
    div_op = mybir.AluOpType.divide
    L = plan.slot_pad[s]
    nk = plan.regions[s] // P
    groups = [(0, 2), (2, nk - 2)]
    attns = []
    for hh in range(2):
        h = 2 * pr + hh
        hp = 64 * (h % 2)
        attn = asb.tile([P, 4, D_HALF], dt.float8e4, tag="attn", name="attn")
        attns.append(attn)
        for pi in range((nk + 1) // 2):
            npl = min(2, nk - 2 * pi)
            sc = scp.tile([P, 2, D_HALF], dt.float32, tag="sc", name="sc")
            # scores feed the exp stream (the global bottleneck): they must
            # preempt bc/V/proj matmuls in the PE queue
            with tc.high_priority():
                for kj in range(npl):
                    ki = 2 * pi + kj
                    nc.tensor.matmul(
                        sc[:, kj, 0:L],
                        kt[s][hp:hp + 64, 0:2, pr, P * ki:P * ki + P],
                        qt[s][hp:hp + 64, 0:2, pr, 0:L],
                        start=True,
                        stop=True,
                        perf_mode=DR,
                    )
            # exp; last chunk gets the per-(core,slot) pad-row bias
            nv = npl - 1 if 2 * pi + npl == nk else npl
            if nv > 0:
                nc.scalar.activation(
                    attn[:, 2 * pi:2 * pi + nv, 0:L],
                    sc[:, 0:nv, 0:L],
                    Exp, scale=EXP_SCALE,
                )
            if nv < npl:
                nc.scalar.activation(
                    attn[:, nk - 1, 0:L],
                    sc[:, npl - 1, 0:L],
                    Exp, scale=EXP_SCALE, bias=bias_sb[:, s:s + 1],
                )
    for hh in range(2):
        h = 2 * pr + hh
        hp = 64 * (h % 2)
        attn = attns[hh]
        otbc = scp.tile([P, 2, D_HALF], dt.float32, tag="sc", name="otbc")
        ot = otbc[:, 0, :]
        bc = otbc[:, 1, :]
        for gi, (k0, npl) in enumerate(groups):
            first, last = gi == 0, gi == len(groups) - 1
            if npl == 2:
                nc.tensor.matmul(
                    bc[:, 0:L], ones_sb[:, :, :],
                    attn[:, k0:k0 + 2, 0:L],
                    start=first, stop=last, perf_mode=DR,
                )
                nc.tensor.matmul(
                    ot[:, 0:L],
                    vv[s][:, k0:k0 + 2, h * P:(h + 1) * P],
                    attn[:, k0:k0 + 2, 0:L],
                    start=first, stop=last, perf_mode=DR,
                )
            else:
                nc.tensor.matmul(
                    bc[:, 0:L], ones_sb[:, 0, :],
                    attn[:, k0, 0:L],
                    start=first, stop=last,
                )
                nc.tensor.matmul(
                    ot[:, 0:L],
                    vv[s][:, k0, h * P:(h + 1) * P],
                    attn[:, k0, 0:L],
                    start=first, stop=last,
                )
        # normalize + distribute. HW: gpsimd can't touch PSUM, and ops may
        # read only ONE input from PSUM -> reciprocal(PSUM->SBUF) on DVE,
        # then two PSUM x SBUF multiplies into the proj-ready layout.
        rec_sb = env["recp"].tile([P, D_HALF], dt.float32, tag="rec",
                                  name="rec")
        with tc.high_priority():
            nc.vector.reciprocal(rec_sb[:, 0:L], bc[:, 0:L])
            nc.vector.tensor_tensor(
                oo[s][hp:hp + 64, 0, pr, 0:L],
                ot[0:64, 0:L], rec_sb[0:64, 0:L], mybir.AluOpType.mult,
            )
            nc.vector.tensor_tensor(
                oo[s][hp:hp + 64, 1, pr, 0:L],
                ot[64:128, 0:L], rec_sb[64:128, 0:L], mybir.AluOpType.mult,
            )


def _proj_ln_closures(nc, tc, plan, env, s, pools):
    scp, xsb, zsb, osb, scrp, stp = pools
    dt = mybir.dt
    oo = env["oo"]
    pw_sb = env["pw_sb"]
    x_d, out_d = env["x_d"], env["out_d"]
    add_op = mybir.AluOpType.add
    sub_op = mybir.AluOpType.subtract
    mult_op = mybir.AluOpType.mult
    if True:
        if True:
            gw = plan.regions[s]
            g0 = plan.offs[s]
            nt = gw // P
            tb = g0 // P  # global tile base
            zs, sq, zt_all = env["zs_g"], env["sq_g"], env["zt_g"]
            fns = []

            def emit_tile(tt):
                lt = tt * P
                t0 = g0 + lt
                x_sb = env["x_slot"][s]
                zp = scp.tile([P, 2, D_HALF], dt.float32, tag="sc", name="zp")
                # deprioritized: proj matmuls fill PE gaps; they must never
                # delay attention scores
                with tc.high_priority(offset=-(10 ** 6)):
                    for i in range(2):
                        for jp in range(2):
                            nc.tensor.matmul(
                                zp[:, i, :],
                                oo[s][:, i, 2 * jp:2 * jp + 2, lt:lt + P],
                                pw_sb[:, i, jp, :, :],
                                start=(jp == 0),
                                stop=(jp == 1),
                                perf_mode=DR,
                            )
                z = zsb.tile([P, 2, D_HALF], dt.bfloat16, tag="z", name="z")
                # z stays scaled by X_SCALE; the layernorm is scale-invariant
                # once eps/rstd constants absorb the factor
                nc.vector.tensor_tensor(
                    z[:, :, :], zp[:, :, :], x_sb[:, tt, :, :], add_op,
                )
                zf = z.rearrange("p a b -> p (a b)")
                scr = scrp.tile([P, D_MODEL], dt.bfloat16, tag="scr",
                                name="scr")
                scr2 = scrp.tile([P, D_MODEL], dt.bfloat16, tag="scr2",
                                 name="scr2")
                # deprioritized: fill ACT gaps in the exp stream / tail
                # (DVE is the busier engine; both row-sums ride on ACT)
                with tc.high_priority(offset=-(10 ** 6)):
                    nc.scalar.activation(
                        scr2[:, :], zf[:, :],
                        mybir.ActivationFunctionType.Copy,
                        accum_out=zs[:, tb + tt:tb + tt + 1],
                    )
                    nc.scalar.activation(
                        scr[:, :], zf[:, :],
                        mybir.ActivationFunctionType.Square,
                        accum_out=sq[:, tb + tt:tb + tt + 1],
                    )
                zt_all[tb + tt] = z

            def emit_xdma():
                x_sb = xsb.tile([P, nt, 2, D_HALF], dt.bfloat16,
                                tag=f"xf{s}", name="x_sb", bufs=1)
                env["x_slot"][s] = x_sb
                nc.sync.dma_start(
                    x_sb.rearrange("p t a b -> p t (a b)")[:, :, :],
                    x_d.rearrange("(t p) d -> p t d", p=P)[:, tb:tb + nt, :])

            fns.append(emit_xdma)
            for tt in range(nt):
                fns.append(lambda tt=tt: emit_tile(tt))
            return fns


def _final_stats(nc, tc, plan, env, pools, c0, c1):
    scp, xsb, zsb, osb, scrp, stp = pools
    dt = mybir.dt
    out_d = env["out_d"]
    add_op = mybir.AluOpType.add
    sub_op = mybir.AluOpType.subtract
    mult_op = mybir.AluOpType.mult
    nt = c1 - c0
    zs_g, sq_g, zt_all = env["zs_g"], env["sq_g"], env["zt_g"]
    zs = zs_g[:, c0:c1]
    sq = sq_g[:, c0:c1]
    if True:
        if True:
            if True:
                # one batched stats chain for ALL 14 tiles. sigma =
                # v*rsqrt(v) with rsqrt by Newton from y0=1 (v is within
                # ~15% of 1 for real tokens) -- no ACT table reloads.
                va = stp.tile([P, nt], dt.float32, tag="va", name="va")
                sig = stp.tile([P, nt], dt.float32, tag="sig", name="sig")
                rstd = stp.tile([P, nt], dt.float32, tag="rstd", name="rstd")
                negmu = stp.tile([P, nt], dt.float32, tag="negmu",
                                 name="negmu")
                y = stp.tile([P, nt], dt.float32, tag="y", name="y")
                t2 = stp.tile([P, nt], dt.float32, tag="t2", name="t2")
                nc.vector.tensor_tensor(va[:, :], zs[:, :], zs[:, :], mult_op)
                nc.vector.tensor_scalar(va[:, :], va[:, :], 1.0 / D_MODEL,
                                        None, mult_op)
                nc.vector.tensor_tensor(va[:, :], sq[:, :], va[:, :], sub_op)
                # v = var/1023, rescaled out of the X_SCALE^2 units
                # (+floor: all-pad token rows have v == 0)
                nc.vector.tensor_scalar(va[:, :], va[:, :],
                                        1.0 / ((D_MODEL - 1) * X_SCALE ** 2),
                                        1e-12, mult_op, add_op)
                nc.gpsimd.tensor_scalar(y[:, :], va[:, :], -0.5, 1.5,
                                        mult_op, add_op)
                for _ in range(2):
                    nc.gpsimd.tensor_tensor(t2[:, :], y[:, :], y[:, :],
                                            mult_op)
                    nc.gpsimd.tensor_tensor(t2[:, :], va[:, :], t2[:, :],
                                            mult_op)
                    nc.gpsimd.tensor_scalar(t2[:, :], t2[:, :], -0.5, 1.5,
                                            mult_op, add_op)
                    nc.gpsimd.tensor_tensor(y[:, :], y[:, :], t2[:, :],
                                            mult_op)
                nc.gpsimd.tensor_tensor(sig[:, :], va[:, :], y[:, :], mult_op)
                # sig holds sqrt(v/X_SCALE^2); z is X_SCALE-scaled, so
                # rstd = 1/(X_SCALE*(sigma + eps))
                nc.vector.tensor_scalar(sig[:, :], sig[:, :], X_SCALE,
                                        EPS * X_SCALE, mult_op, add_op)
                nc.vector.reciprocal(rstd[:, :], sig[:, :])
                nc.vector.tensor_scalar(negmu[:, :], zs[:, :],
                                        -1.0 / D_MODEL, None, mult_op)
                for tt in range(nt):
                    gt = c0 + tt
                    t0 = gt * P
                    o = osb.tile([P, D_MODEL], dt.bfloat16, tag="o", name="o")
                    nc.vector.tensor_scalar(
                        o[:, :],
                        zt_all[gt].rearrange("p a b -> p (a b)")[:, :],
                        negmu[:, tt:tt + 1], rstd[:, tt:tt + 1],
                        add_op, mult_op,
                    )
                    out_eng = nc.gpsimd if tt % 2 == 0 else nc.sync
                    out_eng.dma_start(out_d[t0:t0 + P, :], o[:, :])


_PROGRAMS = {}   # plan.key -> nc
_RUNNERS = {}    # plan.key -> callable


def _get_program(plan: Plan):
    if plan.key not in _PROGRAMS:
        _PROGRAMS[plan.key] = _build_program(plan)
    return _PROGRAMS[plan.key]


def _make_runner(nc, donate=True):
    """Cached PJRT runner (reuses the jitted executable across calls)."""
    import jax
    from jax.sharding import Mesh, PartitionSpec
    from jax.experimental.shard_map import shard_map
    from concourse import bass2jax

    bass2jax.install_neuronx_cc_hook()

    partition_name = (nc.partition_id_tensor.name
                      if nc.partition_id_tensor else None)
    in_names, out_names, out_avals, zero_shapes = [], [], [], []
    for alloc in nc.m.functions[0].allocations:
        if not isinstance(alloc, mybir.MemoryLocationSet):
            continue
        name = alloc.memorylocations[0].name
        if alloc.kind == "ExternalInput":
            if name == partition_name:
                continue
            in_names.append(name)
        elif alloc.kind == "ExternalOutput":
            out_names.append(name)
            shape = tuple(alloc.tensor_shape)
            dtype = mybir.dt.np(alloc.dtype)
            out_avals.append(jax.core.ShapedArray(shape, dtype))
            zero_shapes.append((shape, dtype))
    n_params = len(in_names)
    all_names = in_names + out_names
    if partition_name is not None:
        all_names = all_names + [partition_name]

    def _body(*args):
        operands = list(args)
        if partition_name is not None:
            operands.append(bass2jax.partition_id_tensor())
        outs = bass2jax._bass_exec_p.bind(
            *operands,
            out_avals=tuple(out_avals),
            in_names=tuple(all_names),
            out_names=tuple(out_names),
            lowering_input_output_aliases=(),
            sim_require_finite=True,
            sim_require_nnan=True,
            nc=nc,
        )
        return tuple(outs)

    devices = jax.devices()[:N_CORES]
    mesh = Mesh(np.asarray(devices), ("core",))
    in_specs = (PartitionSpec("core"),) * (n_params + len(out_names))
    out_specs = (PartitionSpec("core"),) * len(out_names)
    sharded = jax.jit(
        shard_map(_body, mesh=mesh, in_specs=in_specs, out_specs=out_specs,
                  check_rep=False),
        donate_argnums=tuple(range(n_params, n_params + len(out_names)))
        if donate else (),
        keep_unused=True,
    )

    def run(in_maps):
        concat_in = [
            np.concatenate([np.asarray(m[name]) for m in in_maps], axis=0)
            for name in in_names
        ]
        concat_zeros = [
            np.zeros((N_CORES * s[0], *s[1:]), d) for (s, d) in zero_shapes
        ]
        out_arrs = sharded(*concat_in, *concat_zeros)
        return [
            {
                name: np.asarray(out_arrs[i]).reshape(
                    N_CORES, *out_avals[i].shape)[c]
                for i, name in enumerate(out_names)
            }
            for c in range(N_CORES)
        ]

    run.sharded = sharded
    run.in_names = in_names
    run.out_names = out_names
    run.out_avals = out_avals
    run.zero_shapes = zero_shapes
    run.n_params = n_params
    return run


def _prep_weights(w_qs1, w_ks1, w_vs1, w_qs2, w_ks2, w_vs2, proj1_w, proj2_w):
    wq = np.zeros((4, 8, P, P), F32)
    wk = np.zeros((4, 8, P, P), F32)
    for pr in range(4):
        h0, h1 = 2 * pr, 2 * pr + 1
        for j in range(8):
            if j < 4:
                rows = slice(j * P, (j + 1) * P)
                wq[pr, j] = np.concatenate(
                    [w_qs1[h0, rows, :], w_qs1[h1, rows, :]], axis=1)
                wk[pr, j] = np.concatenate(
                    [w_ks1[h0, rows, :], w_ks1[h1, rows, :]], axis=1)
            else:
                rows = slice((j - 4) * P, (j - 3) * P)
                wq[pr, j] = np.concatenate(
                    [w_qs2[h0, rows, :], w_qs2[h1, rows, :]], axis=1)
                wk[pr, j] = np.concatenate(
                    [w_ks2[h0, rows, :], w_ks2[h1, rows, :]], axis=1)
    wv = np.zeros((8, P, D_HALF), F32)
    for j in range(8):
        src = w_vs1 if j < 4 else w_vs2
        rows = slice((j % 4) * P, (j % 4 + 1) * P)
        wv[j] = np.concatenate([src[h, rows, :] for h in range(8)], axis=1)
    pw = np.zeros((2, 2, 2, P, D_HALF), F32)
    p1T = np.ascontiguousarray(proj1_w.T)
    p2T = np.ascontiguousarray(proj2_w.T)
    for jp in range(2):
        for t in range(2):
            k = 2 * jp + t
            pw[0, jp, t] = p1T[k * P:(k + 1) * P, :]
            pw[1, jp, t] = p2T[k * P:(k + 1) * P, :]
    # partition-major packing, x W_SCALE, fp8
    wq8 = np.ascontiguousarray(
        (wq * W_SCALE).transpose(2, 0, 1, 3).reshape(P, -1)).astype(FP8)
    wk8 = np.ascontiguousarray(
        (wk * W_SCALE).transpose(2, 0, 1, 3).reshape(P, -1)).astype(FP8)
    wv8 = np.ascontiguousarray(
        (wv * W_SCALE).transpose(1, 0, 2).reshape(P, -1)).astype(FP8)
    pw8 = np.ascontiguousarray(
        (pw * W_SCALE).transpose(3, 0, 1, 2, 4).reshape(P, -1)).astype(FP8)
    return wq8, wk8, wv8, pw8


def _prep_core_inputs(plan: Plan, inp, c):
    T = plan.t_pad
    x = np.zeros((T, D_MODEL), F32)
    bias = np.zeros((P, 4), F32)
    for j in range(4):
        s = plan.core_sents[c][j]
        Lc = int(plan.lengths[s])
        g0 = int(plan.glob_off[s])
        x[plan.offs[j]:plan.offs[j] + Lc] = inp[g0:g0 + Lc]
        nk = plan.regions[j] // P
        nvalid = Lc - P * (nk - 1)
        bias[:, j] = np.where(np.arange(P) < nvalid, 0.0, -30.0)
    xT = np.ascontiguousarray(x.T).astype(FP8)
    return (x * X_SCALE).astype(BF16), xT, bias


def make_in_maps(plan: Plan, inp, weights):
    wq, wk, wv, pw = weights
    in_maps = []
    for c in range(N_CORES):
        x, xT, bias = _prep_core_inputs(plan, inp, c)
        in_maps.append({
            "xT": xT, "x": x, "wq": wq, "wk": wk, "wv": wv, "pw": pw,
            "bias": bias,
        })
    return in_maps


def gather_output(plan: Plan, results, a_2=None, b_2=None):
    T_tot = int(plan.lengths.sum())
    out = np.empty((T_tot, D_MODEL), F32)
    for c in range(N_CORES):
        oc = np.asarray(results[c]["out"], F32)
        for j in range(4):
            s = plan.core_sents[c][j]
            L = int(plan.lengths[s])
            g0 = int(plan.glob_off[s])
            out[g0:g0 + L] = oc[plan.offs[j]:plan.offs[j] + L]
    if a_2 is not None and (np.any(a_2 != 1.0) or np.any(b_2 != 0.0)):
        out = out * np.asarray(a_2, F32) + np.asarray(b_2, F32)
    return out


def kernel(inp, w_qs1, w_ks1, w_vs1, w_qs2, w_ks2, w_vs2,
           proj1_w, proj2_w, a_2, b_2, token_batch, token_pos, valid_mask):
    inp = np.asarray(inp, F32)
    token_batch = np.asarray(token_batch)
    lengths = np.bincount(token_batch, minlength=MB).astype(np.int64)
    plan = Plan(lengths)

    nc = _get_program(plan)
    if plan.key not in _RUNNERS:
        _RUNNERS[plan.key] = _make_runner(nc)
    runner = _RUNNERS[plan.key]

    weights = _prep_weights(np.asarray(w_qs1), np.asarray(w_ks1),
                            np.asarray(w_vs1), np.asarray(w_qs2),
                            np.asarray(w_ks2), np.asarray(w_vs2),
                            np.asarray(proj1_w), np.asarray(proj2_w))
    in_maps = make_in_maps(plan, inp, weights)
    results = runner(in_maps)
    return gather_output(plan, results, np.asarray(a_2), np.asarray(b_2))


# revision 94
# speedup vs baseline: 1.0166x; 1.0166x over previous
# Trainium2 Bass kernel for nn_MultiHeadAttention_24902220382931.
#
# Strategy: data-parallel over sentences. The 32 variable-length sentences are
# sorted by length; core c processes ranks {c, 15-c, 16+c, 31-c} (exactly equal
# token counts, near-equal attention work). Each core packs its 4 sentences
# into 4 fixed-size slots (max length per slot across cores, regions rounded to
# 128) so that all 8 cores execute one identical SPMD program.
#
# v2: fp8(e4m3) DoubleRow matmuls everywhere on PE (2 contraction planes per
# instruction), transposed-attention dataflow:
#   scoresT[k,q] = K^T-chunk @ Q  (dk=128 contracted as 2x64 DoubleRow planes)
#   attn = exp(scoresT*scale + bias_row)    (pad-key rows get bias -30 -> 0)
#   bc[p,q] = ones^T @ attn = sum_k attn    (denominator, broadcast to 128
#                                            partitions for free by the ones
#                                            lhsT having 128 columns)
#   ot[d,q] = V^T-chunks @ attn             (unnormalized output)
#   oo = ot / bc                            (DVE/Pool divide, writes the
#                                            proj-ready layout directly)
# This removes the attn^T transpose matmuls, the per-q-chunk diag build, the
# PSUM->SBUF attn copies and the softmax-sum fixup chain of v1.
#
# Scaling for fp8 dynamic range: all weights are scaled by 32 on the host
# (xavier std ~0.008 would be subnormal in e4m3). logits come out x1024 ->
# exp scale 1/(32*1024). v = 32*v_true -> oo = ot/bc = 32*o_true (good fp8
# range ~0.3). proj weights x32 -> z_psum = 1024*z_true; the residual x is
# pre-scaled x1024 on the host and the z-add applies a 1/1024 output scale.
import sys

for _p in ("/opt/trn_rl_repo", "/root/.axon_site/_ro/trn_rl_repo"):
    if _p not in sys.path:
        sys.path.insert(0, _p)

import numpy as np
import ml_dtypes

import concourse.bass as bass  # noqa: F401
import concourse.mybir as mybir
import concourse.tile as tile
from concourse import bacc

BF16 = ml_dtypes.bfloat16
FP8 = ml_dtypes.float8_e4m3
F32 = np.float32

N_CORES = 8
MB = 32
D_MODEL = 1024
D_HALF = 512
N_HEAD = 8
D_K = 128
SCALE = float(D_MODEL) ** 0.5  # 32.0
W_SCALE = 32.0                 # host-side fp8 weight scale
X_SCALE = 1024.0               # host-side residual scale (W_SCALE**2)
EXP_SCALE = 1.0 / (SCALE * W_SCALE * W_SCALE)
EPS = 1e-3
P = 128


def _ceil_to(x, m):
    return (x + m - 1) // m * m


class Plan:
    def __init__(self, lengths):
        lengths = np.asarray(lengths, np.int64)
        assert lengths.shape == (MB,)
        order = np.argsort(-lengths, kind="stable")
        self.core_sents = [
            [int(order[c]), int(order[15 - c]), int(order[16 + c]), int(order[31 - c])]
            for c in range(N_CORES)
        ]
        self.lengths = lengths
        self.slot_pad = [
            max(int(lengths[self.core_sents[c][j]]) for c in range(N_CORES))
            for j in range(4)
        ]
        self.regions = [_ceil_to(sp, P) for sp in self.slot_pad]
        self.offs = [0]
        for r in self.regions[:-1]:
            self.offs.append(self.offs[-1] + r)
        self.t_pad = sum(self.regions)
        assert self.t_pad % P == 0
        self.nt = self.t_pad // P
        self.glob_off = np.concatenate([[0], np.cumsum(lengths)[:-1]]).astype(np.int64)
        # all chunks below the last must be fully valid on every core
        for s in range(4):
            nk = self.regions[s] // P
            min_l = min(int(lengths[self.core_sents[c][s]]) for c in range(N_CORES))
            assert min_l >= P * (nk - 1), (s, min_l, nk)

    @property
    def key(self):
        return (tuple(self.slot_pad), self.t_pad)


DR = mybir.MatmulPerfMode.DoubleRow


def _build_program(plan: Plan, loop_n: int = 1):
    """Build and compile the single-core Bass program (same for all cores)."""
    import contextlib
    T = plan.t_pad
    nc = bacc.Bacc("TRN2", target_bir_lowering=False, debug=False)

    dt = mybir.dt
    # ---- DRAM I/O ----
    xT_d = nc.dram_tensor("xT", [D_MODEL, T], dt.float8e4, kind="ExternalInput").ap()
    x_d = nc.dram_tensor("x", [T, D_MODEL], dt.bfloat16, kind="ExternalInput").ap()
    wq_d = nc.dram_tensor("wq", [P, 4 * 8 * P], dt.float8e4, kind="ExternalInput").ap()
    wk_d = nc.dram_tensor("wk", [P, 4 * 8 * P], dt.float8e4, kind="ExternalInput").ap()
    wv_d = nc.dram_tensor("wv", [P, 8 * D_HALF], dt.float8e4, kind="ExternalInput").ap()
    pw_d = nc.dram_tensor("pw", [P, 8 * D_HALF], dt.float8e4, kind="ExternalInput").ap()
    bias_d = nc.dram_tensor("bias", [P, 4], dt.float32, kind="ExternalInput").ap()
    out_d = nc.dram_tensor("out", [T, D_MODEL], dt.bfloat16,
                           kind="ExternalOutput").ap()

    with tile.TileContext(nc) as tc:
        with (
            tc.tile_pool(name="persist", bufs=1) as pp,
            tc.tile_pool(name="weights", bufs=1) as wp,
        ):
            # qt/kt: [p(2 heads x 64 dk-of-comp), comp, pair, region] fp8
            qt = [pp.tile([P, 2, 4, r], dt.float8e4, name=f"qt{s}", tag=f"qt{s}")
                  for s, r in enumerate(plan.regions)]
            kt = [pp.tile([P, 2, 4, r], dt.float8e4, name=f"kt{s}", tag=f"kt{s}")
                  for s, r in enumerate(plan.regions)]
            # V token-natural: [p, tile, head*128 + half*64 + d]
            vv = [pp.tile([P, r // P, D_MODEL], dt.float8e4, name=f"vv{s}",
                          tag=f"vv{s}")
                  for s, r in enumerate(plan.regions)]
            # attention out, proj-ready: [64*(h%2)+d, i(v1/v2), h//2, token]
            oo = [pp.tile([P, 2, 4, r], dt.float8e4, name=f"oo{s}",
                          tag=f"oo{s}")
                  for s, r in enumerate(plan.regions)]
            bias_sb = pp.tile([P, 4], dt.float32, tag="bias")
            ones_sb = pp.tile([P, 2, P], dt.float8e4, tag="ones")

            wq_sb = wp.tile([P, 4, 8, P], dt.float8e4, tag="wq")
            wk_sb = wp.tile([P, 4, 8, P], dt.float8e4, tag="wk")
            wv_sb = wp.tile([P, 8, D_HALF], dt.float8e4, tag="wv")
            pw_sb = wp.tile([P, 2, 2, 2, D_HALF], dt.float8e4, tag="pw")

            nc.gpsimd.dma_start(bias_sb[:, :], bias_d[:, :])
            nc.scalar.dma_start(
                wq_sb.rearrange("p a b c -> p (a b c)")[:, :], wq_d[:, :])
            nc.scalar.dma_start(
                wk_sb.rearrange("p a b c -> p (a b c)")[:, :], wk_d[:, :])
            nc.gpsimd.dma_start(
                wv_sb.rearrange("p a b -> p (a b)")[:, :], wv_d[:, :])
            nc.scalar.dma_start(
                pw_sb.rearrange("p a b c d -> p (a b c d)")[:, :], pw_d[:, :])
            nc.gpsimd.memset(ones_sb[:, :, :], 1.0)

            loop_cm = (tc.For_i(0, loop_n, 1,
                                hint_engines=(mybir.EngineType.PE,
                                              mybir.EngineType.DVE,
                                              mybir.EngineType.Activation,
                                              mybir.EngineType.SP))
                       if loop_n > 1 else contextlib.nullcontext())
            with loop_cm:
                _kernel_body(nc, tc, plan, locals())

    nc.compile()
    return nc


def _kernel_body(nc, tc, plan, env):
    dt = mybir.dt
    qt, kt, vv, oo = env["qt"], env["kt"], env["vv"], env["oo"]
    bias_sb, ones_sb = env["bias_sb"], env["ones_sb"]
    wq_sb, wk_sb, wv_sb, pw_sb = (env["wq_sb"], env["wk_sb"], env["wv_sb"],
                                  env["pw_sb"])
    xT_d, x_d, out_d = env["xT_d"], env["x_d"], env["out_d"]
    Exp = mybir.ActivationFunctionType.Exp
    Square = mybir.ActivationFunctionType.Square
    Sqrt = mybir.ActivationFunctionType.Sqrt
    add_op = mybir.AluOpType.add
    sub_op = mybir.AluOpType.subtract
    mult_op = mybir.AluOpType.mult
    div_op = mybir.AluOpType.divide

    # zero the pad-query tails of the attention output staging (never written
    # by normalize; proj matmuls read the full region)
    for s in range(4):
        L, r = plan.slot_pad[s], plan.regions[s]
        if r > L:
            nc.gpsimd.memset(oo[s][:, :, :, L:r], 0.0)

    # ============ QKV projections, then per-slot attention + proj/LN =======
    # PSUM budget (8 banks): sc pool = 3 x [128,2,512] tiles (6 banks) through
    # which scores pairs, the ot/bc combo AND the proj z accumulators all
    # rotate; acc pool (QKV) = 2 banks.
    with (
        tc.tile_pool(name="xt_pool", bufs=4) as xtp,
        tc.tile_pool(name="sc_ps", bufs=3, space="PSUM") as scp,
        tc.tile_pool(name="attn_sb", bufs=3) as asb,
        tc.tile_pool(name="rec_sb", bufs=2) as recp,
    ):
        env["recp"] = recp
        # ---- QKV ----
        ev_rot = 0
        ev_engs = (nc.vector, nc.scalar)
        with tc.tile_pool(name="acc_ps", bufs=2, space="PSUM") as accp:
            for s in range(4):
                gw = plan.regions[s]
                g0 = plan.offs[s]
                xt_sb = xtp.tile([P, 8, D_HALF], dt.float8e4, tag="xt")
                xT_r = xT_d.rearrange("(c p) t -> p c t", p=P)
                nc.sync.dma_start(xt_sb[:, :, 0:gw], xT_r[:, :, g0:g0 + gw])
                # pr0 Q/K first (unblocks heads 0-1's scores), then all of V
                # (every head's attn@V needs the whole slot's vv), then the
                # remaining Q/K pairs
                def emit_qk(pr):
                    for w_sb, dst in ((wq_sb, qt), (wk_sb, kt)):
                        # scores read only query cols 0:L of qt; kt needs the
                        # full region (whole key chunks)
                        cw = plan.slot_pad[s] if dst is qt else gw
                        for half in range(2):
                            acc = accp.tile([P, D_HALF], dt.float32,
                                            tag="acc", name="acc")
                            for jp in range(2):
                                j = half * 4 + jp * 2
                                nc.tensor.matmul(
                                    acc[:, 0:cw],
                                    w_sb[:, pr, j:j + 2, :],
                                    xt_sb[:, j:j + 2, 0:cw],
                                    start=(jp == 0),
                                    stop=(jp == 1),
                                    perf_mode=DR,
                                )
                            nonlocal ev_rot
                            eng = ev_engs[ev_rot % len(ev_engs)]
                            ev_rot += 1
                            off = None if eng is nc.vector else -(10 ** 6)
                            with tc.high_priority(offset=off):
                                if eng is nc.scalar:
                                    eng.copy(dst[s][:, half, pr, 0:cw],
                                             acc[:, 0:cw])
                                else:
                                    eng.tensor_copy(dst[s][:, half, pr, 0:cw],
                                                    acc[:, 0:cw])

                emit_qk(0)
                vv_w = vv[s].rearrange("p t (h b d) -> p t h b d",
                                       h=N_HEAD, b=2)
                for tt in range(gw // P):
                    tl = tt * P
                    for half in range(2):
                        vacc = accp.tile([P, D_HALF], dt.float32,
                                         tag="acc", name="vacc")
                        vacc_r = vacc.rearrange("p (h d) -> p h d", h=N_HEAD)
                        for jp in range(2):
                            j = half * 4 + jp * 2
                            nc.tensor.matmul(
                                vacc[:, :],
                                xt_sb[:, j:j + 2, tl:tl + P],
                                wv_sb[:, j:j + 2, :],
                                start=(jp == 0),
                                stop=(jp == 1),
                                perf_mode=DR,
                            )
                        eng = ev_engs[ev_rot % len(ev_engs)]
                        ev_rot += 1
                        off = None if eng is nc.vector else -(10 ** 6)
                        with tc.high_priority(offset=off):
                            if eng is nc.scalar:
                                eng.copy(vv_w[:, tt, :, half, :],
                                         vacc_r[:, :, :])
                            else:
                                eng.tensor_copy(vv_w[:, tt, :, half, :],
                                                vacc_r[:, :, :])
                for pr in range(1, 4):
                    emit_qk(pr)

        # ---- per slot: attention heads with proj tiles of the previous
        # slot interleaved between head pairs (they share the sc rotation) --
        with (
            tc.tile_pool(name="x_sb", bufs=2) as xsb,
            tc.tile_pool(name="z_sb", bufs=16) as zsb,
            tc.tile_pool(name="o_sb", bufs=2) as osb,
            tc.tile_pool(name="scr_sb", bufs=2) as scrp,
            tc.tile_pool(name="st_sb", bufs=2) as stp,
        ):
            pools = (scp, xsb, zsb, osb, scrp, stp)
            env["zs_g"] = stp.tile([P, plan.nt], mybir.dt.float32,
                                   tag="zs", bufs=1, name="zs_g")
            env["sq_g"] = stp.tile([P, plan.nt], mybir.dt.float32,
                                   tag="sq", bufs=1, name="sq_g")
            env["zt_g"] = [None] * plan.nt
            env["x_slot"] = [None] * 4
            prev = []
            for s in range(4):
                it = iter(prev)
                for pr in range(4):
                    _attn_pair(nc, tc, plan, env, s, pr, scp, asb)
                    fn = next(it, None)
                    if fn is not None:
                        fn()
                for fn in it:
                    fn()
                prev = _proj_ln_closures(nc, tc, plan, env, s, pools)
                if s == 2:
                    # slots 0-1 proj tiles have drained; their stats +
                    # applies overlap slot-3 attention
                    _final_stats(nc, tc, plan, env, pools, 0,
                                 plan.offs[2] // P)
            for fn in prev:
                fn()
            _final_stats(nc, tc, plan, env, pools,
                         plan.offs[2] // P, plan.offs[3] // P)
            _final_stats(nc, tc, plan, env, pools,
                         plan.offs[3] // P, plan.nt)


def _attn_pair(nc, tc, plan, env, s, pr, scp, asb):
    """Attention for the head pair (2*pr, 2*pr+1) of slot s.

    scores/exp run per head; the attn@V and denominator matmuls write M=64
    groups into shared pair tiles so ONE [128, 2, L] DVE divide normalizes
    and distributes both heads (GPSIMD may not touch PSUM on real HW)."""
    dt = mybir.dt
    qt, kt, vv, oo = env["qt"], env["kt"], env["vv"], env["oo"]
    bias_sb, ones_sb = env["bias_sb"], env["ones_sb"]
    Exp = mybir.ActivationFunctionType.Exp
    div_op = mybir.AluOpType.divide
    L = plan.slot_pad[s]
    nk = plan.regions[s] // P
    groups = [(0, 2), (2, nk - 2)]
    attns = []
    for hh in range(2):
        h = 2 * pr + hh
        hp = 64 * (h % 2)
        attn = asb.tile([P, 4, D_HALF], dt.float8e4, tag="attn", name="attn")
        attns.append(attn)
        for pi in range((nk + 1) // 2):
            npl = min(2, nk - 2 * pi)
            sc = scp.tile([P, 2, D_HALF], dt.float32, tag="sc", name="sc")
            # scores feed the exp stream (the global bottleneck): they must
            # preempt bc/V/proj matmuls in the PE queue
            with tc.high_priority():
                for kj in range(npl):
                    ki = 2 * pi + kj
                    nc.tensor.matmul(
                        sc[:, kj, 0:L],
                        kt[s][hp:hp + 64, 0:2, pr, P * ki:P * ki + P],
                        qt[s][hp:hp + 64, 0:2, pr, 0:L],
                        start=True,
                        stop=True,
                        perf_mode=DR,
                    )
            # exp; last chunk gets the per-(core,slot) pad-row bias
            nv = npl - 1 if 2 * pi + npl == nk else npl
            if nv > 0:
                nc.scalar.activation(
                    attn[:, 2 * pi:2 * pi + nv, 0:L],
                    sc[:, 0:nv, 0:L],
                    Exp, scale=EXP_SCALE,
                )
            if nv < npl:
                nc.scalar.activation(
                    attn[:, nk - 1, 0:L],
                    sc[:, npl - 1, 0:L],
                    Exp, scale=EXP_SCALE, bias=bias_sb[:, s:s + 1],
                )
    for hh in range(2):
        h = 2 * pr + hh
        hp = 64 * (h % 2)
        attn = attns[hh]
        otbc = scp.tile([P, 2, D_HALF], dt.float32, tag="sc", name="otbc")
        ot = otbc[:, 0, :]
        bc = otbc[:, 1, :]
        for gi, (k0, npl) in enumerate(groups):
            first, last = gi == 0, gi == len(groups) - 1
            if npl == 2:
                nc.tensor.matmul(
                    bc[:, 0:L], ones_sb[:, :, :],
                    attn[:, k0:k0 + 2, 0:L],
                    start=first, stop=last, perf_mode=DR,
                )
                nc.tensor.matmul(
                    ot[:, 0:L],
                    vv[s][:, k0:k0 + 2, h * P:(h + 1) * P],
                    attn[:, k0:k0 + 2, 0:L],
                    start=first, stop=last, perf_mode=DR,
                )
            else:
                nc.tensor.matmul(
                    bc[:, 0:L], ones_sb[:, 0, :],
                    attn[:, k0, 0:L],
                    start=first, stop=last,
                )
                nc.tensor.matmul(
                    ot[:, 0:L],
                    vv[s][:, k0, h * P:(h + 1) * P],
                    attn[:, k0, 0:L],
                    start=first, stop=last,
                )
        # normalize + distribute. HW: gpsimd can't touch PSUM, and ops may
        # read only ONE input from PSUM -> reciprocal(PSUM->SBUF) on DVE,
        # then two PSUM x SBUF multiplies into the proj-ready layout.
        rec_sb = env["recp"].tile([P, D_HALF], dt.float32, tag="rec",
                                  name="rec")
        with tc.high_priority():
            nc.vector.reciprocal(rec_sb[:, 0:L], bc[:, 0:L])
        nc.vector.tensor_tensor(
            oo[s][hp:hp + 64, 0, pr, 0:L],
            ot[0:64, 0:L], rec_sb[0:64, 0:L], mybir.AluOpType.mult,
        )
        nc.vector.tensor_tensor(
            oo[s][hp:hp + 64, 1, pr, 0:L],
            ot[64:128, 0:L], rec_sb[64:128, 0:L], mybir.AluOpType.mult,
        )


def _proj_ln_closures(nc, tc, plan, env, s, pools):
    scp, xsb, zsb, osb, scrp, stp = pools
    dt = mybir.dt
    oo = env["oo"]
    pw_sb = env["pw_sb"]
    x_d, out_d = env["x_d"], env["out_d"]
    add_op = mybir.AluOpType.add
    sub_op = mybir.AluOpType.subtract
    mult_op = mybir.AluOpType.mult
    if True:
        if True:
            gw = plan.regions[s]
            g0 = plan.offs[s]
            nt = gw // P
            tb = g0 // P  # global tile base
            zs, sq, zt_all = env["zs_g"], env["sq_g"], env["zt_g"]
            fns = []

            def emit_tile(tt):
                lt = tt * P
                t0 = g0 + lt
                x_sb = env["x_slot"][s]
                zp = scp.tile([P, 2, D_HALF], dt.float32, tag="sc", name="zp")
                # deprioritized: proj matmuls fill PE gaps; they must never
                # delay attention scores
                with tc.high_priority(offset=-(10 ** 6)):
                    for i in range(2):
                        for jp in range(2):
                            nc.tensor.matmul(
                                zp[:, i, :],
                                oo[s][:, i, 2 * jp:2 * jp + 2, lt:lt + P],
                                pw_sb[:, i, jp, :, :],
                                start=(jp == 0),
                                stop=(jp == 1),
                                perf_mode=DR,
                            )
                z = zsb.tile([P, 2, D_HALF], dt.bfloat16, tag="z", name="z")
                # z stays scaled by X_SCALE; the layernorm is scale-invariant
                # once eps/rstd constants absorb the factor
                nc.vector.tensor_tensor(
                    z[:, :, :], zp[:, :, :], x_sb[:, tt, :, :], add_op,
                )
                zf = z.rearrange("p a b -> p (a b)")
                scr = scrp.tile([P, D_MODEL], dt.bfloat16, tag="scr",
                                name="scr")
                scr2 = scrp.tile([P, D_MODEL], dt.bfloat16, tag="scr2",
                                 name="scr2")
                # deprioritized: fill ACT gaps in the exp stream / tail
                # (DVE is the busier engine; both row-sums ride on ACT)
                with tc.high_priority(offset=-(10 ** 6)):
                    nc.scalar.activation(
                        scr2[:, :], zf[:, :],
                        mybir.ActivationFunctionType.Copy,
                        accum_out=zs[:, tb + tt:tb + tt + 1],
                    )
                    nc.scalar.activation(
                        scr[:, :], zf[:, :],
                        mybir.ActivationFunctionType.Square,
                        accum_out=sq[:, tb + tt:tb + tt + 1],
                    )
                zt_all[tb + tt] = z

            def emit_xdma():
                x_sb = xsb.tile([P, nt, 2, D_HALF], dt.bfloat16,
                                tag=f"xf{s}", name="x_sb", bufs=1)
                env["x_slot"][s] = x_sb
                nc.sync.dma_start(
                    x_sb.rearrange("p t a b -> p t (a b)")[:, :, :],
                    x_d.rearrange("(t p) d -> p t d", p=P)[:, tb:tb + nt, :])

            fns.append(emit_xdma)
            for tt in range(nt):
                fns.append(lambda tt=tt: emit_tile(tt))
            return fns


def _final_stats(nc, tc, plan, env, pools, c0, c1):
    scp, xsb, zsb, osb, scrp, stp = pools
    dt = mybir.dt
    out_d = env["out_d"]
    add_op = mybir.AluOpType.add
    sub_op = mybir.AluOpType.subtract
    mult_op = mybir.AluOpType.mult
    nt = c1 - c0
    zs_g, sq_g, zt_all = env["zs_g"], env["sq_g"], env["zt_g"]
    zs = zs_g[:, c0:c1]
    sq = sq_g[:, c0:c1]
    if True:
        if True:
            if True:
                # one batched stats chain for ALL 14 tiles. sigma =
                # v*rsqrt(v) with rsqrt by Newton from y0=1 (v is within
                # ~15% of 1 for real tokens) -- no ACT table reloads.
                va = stp.tile([P, nt], dt.float32, tag="va", name="va")
                sig = stp.tile([P, nt], dt.float32, tag="sig", name="sig")
                rstd = stp.tile([P, nt], dt.float32, tag="rstd", name="rstd")
                negmu = stp.tile([P, nt], dt.float32, tag="negmu",
                                 name="negmu")
                y = stp.tile([P, nt], dt.float32, tag="y", name="y")
                t2 = stp.tile([P, nt], dt.float32, tag="t2", name="t2")
                nc.vector.tensor_tensor(va[:, :], zs[:, :], zs[:, :], mult_op)
                nc.vector.tensor_scalar(va[:, :], va[:, :], 1.0 / D_MODEL,
                                        None, mult_op)
                nc.vector.tensor_tensor(va[:, :], sq[:, :], va[:, :], sub_op)
                # v = var/1023, rescaled out of the X_SCALE^2 units
                # (+floor: all-pad token rows have v == 0)
                nc.vector.tensor_scalar(va[:, :], va[:, :],
                                        1.0 / ((D_MODEL - 1) * X_SCALE ** 2),
                                        1e-12, mult_op, add_op)
                nc.gpsimd.tensor_scalar(y[:, :], va[:, :], -0.5, 1.5,
                                        mult_op, add_op)
                for _ in range(2):
                    nc.gpsimd.tensor_tensor(t2[:, :], y[:, :], y[:, :],
                                            mult_op)
                    nc.gpsimd.tensor_tensor(t2[:, :], va[:, :], t2[:, :],
                                            mult_op)
                    nc.gpsimd.tensor_scalar(t2[:, :], t2[:, :], -0.5, 1.5,
                                            mult_op, add_op)
                    nc.gpsimd.tensor_tensor(y[:, :], y[:, :], t2[:, :],
                                            mult_op)
                nc.gpsimd.tensor_tensor(sig[:, :], va[:, :], y[:, :], mult_op)
                # sig holds sqrt(v/X_SCALE^2); z is X_SCALE-scaled, so
                # rstd = 1/(X_SCALE*(sigma + eps))
                nc.vector.tensor_scalar(sig[:, :], sig[:, :], X_SCALE,
                                        EPS * X_SCALE, mult_op, add_op)
                nc.vector.reciprocal(rstd[:, :], sig[:, :])
                nc.vector.tensor_scalar(negmu[:, :], zs[:, :],
                                        -1.0 / D_MODEL, None, mult_op)
                for tt in range(nt):
                    gt = c0 + tt
                    t0 = gt * P
                    o = osb.tile([P, D_MODEL], dt.bfloat16, tag="o", name="o")
                    nc.vector.tensor_scalar(
                        o[:, :],
                        zt_all[gt].rearrange("p a b -> p (a b)")[:, :],
                        negmu[:, tt:tt + 1], rstd[:, tt:tt + 1],
                        add_op, mult_op,
                    )
                    out_eng = nc.gpsimd if tt % 2 == 0 else nc.sync
                    out_eng.dma_start(out_d[t0:t0 + P, :], o[:, :])


_PROGRAMS = {}   # plan.key -> nc
_RUNNERS = {}    # plan.key -> callable


def _get_program(plan: Plan):
    if plan.key not in _PROGRAMS:
        _PROGRAMS[plan.key] = _build_program(plan)
    return _PROGRAMS[plan.key]


def _make_runner(nc, donate=True):
    """Cached PJRT runner (reuses the jitted executable across calls)."""
    import jax
    from jax.sharding import Mesh, PartitionSpec
    from jax.experimental.shard_map import shard_map
    from concourse import bass2jax

    bass2jax.install_neuronx_cc_hook()

    partition_name = (nc.partition_id_tensor.name
                      if nc.partition_id_tensor else None)
    in_names, out_names, out_avals, zero_shapes = [], [], [], []
    for alloc in nc.m.functions[0].allocations:
        if not isinstance(alloc, mybir.MemoryLocationSet):
            continue
        name = alloc.memorylocations[0].name
        if alloc.kind == "ExternalInput":
            if name == partition_name:
                continue
            in_names.append(name)
        elif alloc.kind == "ExternalOutput":
            out_names.append(name)
            shape = tuple(alloc.tensor_shape)
            dtype = mybir.dt.np(alloc.dtype)
            out_avals.append(jax.core.ShapedArray(shape, dtype))
            zero_shapes.append((shape, dtype))
    n_params = len(in_names)
    all_names = in_names + out_names
    if partition_name is not None:
        all_names = all_names + [partition_name]

    def _body(*args):
        operands = list(args)
        if partition_name is not None:
            operands.append(bass2jax.partition_id_tensor())
        outs = bass2jax._bass_exec_p.bind(
            *operands,
            out_avals=tuple(out_avals),
            in_names=tuple(all_names),
            out_names=tuple(out_names),
            lowering_input_output_aliases=(),
            sim_require_finite=True,
            sim_require_nnan=True,
            nc=nc,
        )
        return tuple(outs)

    devices = jax.devices()[:N_CORES]
    mesh = Mesh(np.asarray(devices), ("core",))
    in_specs = (PartitionSpec("core"),) * (n_params + len(out_names))
    out_specs = (PartitionSpec("core"),) * len(out_names)
    sharded = jax.jit(
        shard_map(_body, mesh=mesh, in_specs=in_specs, out_specs=out_specs,
                  check_rep=False),
        donate_argnums=tuple(range(n_params, n_params + len(out_names)))
        if donate else (),
        keep_unused=True,
    )

    def run(in_maps):
        concat_in = [
            np.concatenate([np.asarray(m[name]) for m in in_maps], axis=0)
            for name in in_names
        ]
        concat_zeros = [
            np.zeros((N_CORES * s[0], *s[1:]), d) for (s, d) in zero_shapes
        ]
        out_arrs = sharded(*concat_in, *concat_zeros)
        return [
            {
                name: np.asarray(out_arrs[i]).reshape(
                    N_CORES, *out_avals[i].shape)[c]
                for i, name in enumerate(out_names)
            }
            for c in range(N_CORES)
        ]

    run.sharded = sharded
    run.in_names = in_names
    run.out_names = out_names
    run.out_avals = out_avals
    run.zero_shapes = zero_shapes
    run.n_params = n_params
    return run


def _prep_weights(w_qs1, w_ks1, w_vs1, w_qs2, w_ks2, w_vs2, proj1_w, proj2_w):
    wq = np.zeros((4, 8, P, P), F32)
    wk = np.zeros((4, 8, P, P), F32)
    for pr in range(4):
        h0, h1 = 2 * pr, 2 * pr + 1
        for j in range(8):
            if j < 4:
                rows = slice(j * P, (j + 1) * P)
                wq[pr, j] = np.concatenate(
                    [w_qs1[h0, rows, :], w_qs1[h1, rows, :]], axis=1)
                wk[pr, j] = np.concatenate(
                    [w_ks1[h0, rows, :], w_ks1[h1, rows, :]], axis=1)
            else:
                rows = slice((j - 4) * P, (j - 3) * P)
                wq[pr, j] = np.concatenate(
                    [w_qs2[h0, rows, :], w_qs2[h1, rows, :]], axis=1)
                wk[pr, j] = np.concatenate(
                    [w_ks2[h0, rows, :], w_ks2[h1, rows, :]], axis=1)
    wv = np.zeros((8, P, D_HALF), F32)
    for j in range(8):
        src = w_vs1 if j < 4 else w_vs2
        rows = slice((j % 4) * P, (j % 4 + 1) * P)
        wv[j] = np.concatenate([src[h, rows, :] for h in range(8)], axis=1)
    pw = np.zeros((2, 2, 2, P, D_HALF), F32)
    p1T = np.ascontiguousarray(proj1_w.T)
    p2T = np.ascontiguousarray(proj2_w.T)
    for jp in range(2):
        for t in range(2):
            k = 2 * jp + t
            pw[0, jp, t] = p1T[k * P:(k + 1) * P, :]
            pw[1, jp, t] = p2T[k * P:(k + 1) * P, :]
    # partition-major packing, x W_SCALE, fp8
    wq8 = np.ascontiguousarray(
        (wq * W_SCALE).transpose(2, 0, 1, 3).reshape(P, -1)).astype(FP8)
    wk8 = np.ascontiguousarray(
        (wk * W_SCALE).transpose(2, 0, 1, 3).reshape(P, -1)).astype(FP8)
    wv8 = np.ascontiguousarray(
        (wv * W_SCALE).transpose(1, 0, 2).reshape(P, -1)).astype(FP8)
    pw8 = np.ascontiguousarray(
        (pw * W_SCALE).transpose(3, 0, 1, 2, 4).reshape(P, -1)).astype(FP8)
    return wq8, wk8, wv8, pw8


def _prep_core_inputs(plan: Plan, inp, c):
    T = plan.t_pad
    x = np.zeros((T, D_MODEL), F32)
    bias = np.zeros((P, 4), F32)
    for j in range(4):
        s = plan.core_sents[c][j]
        Lc = int(plan.lengths[s])
        g0 = int(plan.glob_off[s])
        x[plan.offs[j]:plan.offs[j] + Lc] = inp[g0:g0 + Lc]
        nk = plan.regions[j] // P
        nvalid = Lc - P * (nk - 1)
        bias[:, j] = np.where(np.arange(P) < nvalid, 0.0, -30.0)
    xT = np.ascontiguousarray(x.T).astype(FP8)
    return (x * X_SCALE).astype(BF16), xT, bias


def make_in_maps(plan: Plan, inp, weights):
    wq, wk, wv, pw = weights
    in_maps = []
    for c in range(N_CORES):
        x, xT, bias = _prep_core_inputs(plan, inp, c)
        in_maps.append({
            "xT": xT, "x": x, "wq": wq, "wk": wk, "wv": wv, "pw": pw,
            "bias": bias,
        })
    return in_maps


def gather_output(plan: Plan, results, a_2=None, b_2=None):
    T_tot = int(plan.lengths.sum())
    out = np.empty((T_tot, D_MODEL), F32)
    for c in range(N_CORES):
        oc = np.asarray(results[c]["out"], F32)
        for j in range(4):
            s = plan.core_sents[c][j]
            L = int(plan.lengths[s])
            g0 = int(plan.glob_off[s])
            out[g0:g0 + L] = oc[plan.offs[j]:plan.offs[j] + L]
    if a_2 is not None and (np.any(a_2 != 1.0) or np.any(b_2 != 0.0)):
        out = out * np.asarray(a_2, F32) + np.asarray(b_2, F32)
    return out


def kernel(inp, w_qs1, w_ks1, w_vs1, w_qs2, w_ks2, w_vs2,
           proj1_w, proj2_w, a_2, b_2, token_batch, token_pos, valid_mask):
    inp = np.asarray(inp, F32)
    token_batch = np.asarray(token_batch)
    lengths = np.bincount(token_batch, minlength=MB).astype(np.int64)
    plan = Plan(lengths)

    nc = _get_program(plan)
    if plan.key not in _RUNNERS:
        _RUNNERS[plan.key] = _make_runner(nc)
    runner = _RUNNERS[plan.key]

    weights = _prep_weights(np.asarray(w_qs1), np.asarray(w_ks1),
                            np.asarray(w_vs1), np.asarray(w_qs2),
                            np.asarray(w_ks2), np.asarray(w_vs2),
                            np.asarray(proj1_w), np.asarray(proj2_w))
    in_maps = make_in_maps(plan, inp, weights)
    results = runner(in_maps)
    return gather_output(plan, results, np.asarray(a_2), np.asarray(b_2))
